# revision 1
# baseline (speedup 1.0000x reference)
"""Trainium2 Bass kernel for nn_BaseModel_14499809591724 (GNN message passing).

Strategy (8 NeuronCores, data-parallel over graph batches):
  - Nodes are split into 8 contiguous shards at graph boundaries (batch is
    sorted), padded to S=6400 rows each; full node table = [8*S, 128].
  - Each core owns the edges whose dst falls in its shard. Edges are sorted by
    (dst window of 128 nodes, src-table half) and chunked into groups of 128.
  - Per GCN conv: gather h[src] rows (bf16) from the replicated DRAM table with
    dma_gather; build a scaled one-hot [e, d] = (d == rel[e]) * norm[e] on DVE;
    scatter-reduce via PE matmul into PSUM per 128-dst window; add the
    self-loop term dinv2*h; apply W + bias + ReLU.
  - After each conv that feeds another conv, the 8 local shards are AllGathered
    (bf16) to rebuild the replicated table.
  - JumpingKnowledge + per-graph pooling (one-hot matmul) + BN + MLP head +
    softmax run per core on its own 64 graphs; host concatenates 8 x [64, 10].

All heavy compute runs on device. Host does index/layout preprocessing and
edge-weight normalization (deg/dinv/norm), which is sharding metadata.
"""
import sys
import numpy as np
import ml_dtypes

sys.path.insert(0, "/opt/trn_rl_repo")

from concourse import bacc, tile, mybir  # noqa: E402
from concourse.bass_utils import run_bass_kernel_spmd  # noqa: E402

# ---- model / sharding constants (shapes fixed by the problem) ----
NC = 8
N_NODES = 50000
N_EDGES = 800000
F = 128
B = 512
GPC = B // NC          # graphs per core = 64
S = 6400               # padded nodes per shard (max real shard is 6368)
NW = S // 128          # 50 windows per core
TAB = NC * S           # 51200 table rows
NCH = 2                # table chunks (progressive AllGather pipeline)
CHS = S // NCH         # 3200 shard rows per chunk
CHROWS = NC * CHS      # 25600 table rows per chunk (int16-safe)
CH = 9                 # 128-edge groups per (window, table-chunk); max seen 1112 edges
CPW = NCH * CH         # 20
NIDX = CH * 128        # 640 idxs per gather
ICOLS = NIDX // 16     # 40 wrapped idx columns per gather
NB = 3
BN_EPS = 1e-5

f32 = mybir.dt.float32
bf16 = mybir.dt.bfloat16
i16 = mybir.dt.int16

_PROGRAM = None
import os
NWRUN = int(os.environ.get("NWRUN", NW))
BUFS1 = os.environ.get("BUFS1") == "1"
SKIP_GATHER = os.environ.get("SKIP_GATHER") == "1"
SKIP_TS = os.environ.get("SKIP_TS") == "1"
REPEAT = int(os.environ.get("REPEAT", "1"))
AGQ = int(os.environ.get("AGQ", "1"))  # AG 1/AGQ of the shard (timing probe)


def _bufs(n):
    return 1 if BUFS1 else n



def _wrap_idxs(runs: np.ndarray) -> np.ndarray:
    """[R, NIDX] int -> [128, R*ICOLS] int16 (16-partition wrap, 8x replicated)."""
    r = runs.shape[0]
    w = runs.reshape(r, ICOLS, 16).transpose(2, 0, 1).reshape(16, r * ICOLS)
    return np.tile(w.astype(np.int16), (8, 1))


def _preprocess(inp: dict):
    batch = np.asarray(inp["batch"])
    ei = np.asarray(inp["edge_index"])
    ew = np.asarray(inp["edge_attr"], dtype=np.float32)
    x = np.asarray(inp["x"], dtype=np.float32)
    src, dst = ei[0].astype(np.int64), ei[1].astype(np.int64)

    bounds = np.searchsorted(batch, np.arange(0, B + 1, GPC)).astype(np.int64)
    sizes = np.diff(bounds)
    assert sizes.max() <= S, f"shard overflow: {sizes.max()} > {S}"

    node = np.arange(N_NODES, dtype=np.int64)
    core_of = (np.searchsorted(bounds, node, side="right") - 1).astype(np.int64)
    off = node - bounds[core_of]
    # chunk-major table: row = chunk*CHROWS + core*CHS + (off % CHS)
    tab = (off // CHS) * CHROWS + core_of * CHS + (off % CHS)

    deg = (np.bincount(dst, weights=ew.astype(np.float64), minlength=N_NODES) + 1.0)
    deg = deg.astype(np.float32)
    dinv = 1.0 / np.sqrt(deg)
    norm = (dinv[src] * ew * dinv[dst]).astype(np.float32)
    dinv2 = (1.0 / deg).astype(np.float32)

    # full replicated x table (node-major, bf16)
    xtab = np.zeros((TAB, F), dtype=ml_dtypes.bfloat16)
    xtab[tab] = x.astype(ml_dtypes.bfloat16)

    iota = np.tile(np.arange(128, dtype=np.float32), (128, 1)).astype(ml_dtypes.bfloat16)
    identf = np.eye(128, dtype=np.float32)
    identb = np.eye(128, dtype=ml_dtypes.bfloat16)

    # weights
    conv_w = np.asarray(inp["conv_w"], dtype=np.float32).reshape(6, F, F)
    convw = conv_w.transpose(1, 0, 2).reshape(F, 6 * F).astype(ml_dtypes.bfloat16)
    convb = np.asarray(inp["conv_b"], dtype=np.float32).reshape(6, F).T.copy()
    jk_w = np.asarray(inp["jk_w"], dtype=np.float32).reshape(NB, 2, F, F).reshape(6, F, F)
    jkw = jk_w.transpose(1, 0, 2).reshape(F, 6 * F).astype(ml_dtypes.bfloat16)
    jkb = np.asarray(inp["jk_b"], dtype=np.float32).T.copy()
    s = (np.asarray(inp["bn_gamma"], dtype=np.float32)
         / np.sqrt(np.asarray(inp["bn_var"], dtype=np.float32) + BN_EPS))
    t = (np.asarray(inp["bn_beta"], dtype=np.float32)
         - np.asarray(inp["bn_mean"], dtype=np.float32) * s)
    bns = s.reshape(NB, F).T.copy()
    bnt = t.reshape(NB, F).T.copy()
    lin1_w = np.asarray(inp["lin1_w"], dtype=np.float32).reshape(NB, F, F)
    l1w = lin1_w.transpose(1, 0, 2).reshape(F, NB * F).copy()
    l1b = np.asarray(inp["lin1_b"], dtype=np.float32).reshape(F, 1).copy()
    l2w = np.asarray(inp["lin2_w"], dtype=np.float32).copy()
    l2b = np.asarray(inp["lin2_b"], dtype=np.float32).reshape(10, 1).copy()

    shared = {
        "xtab": xtab, "iota": iota, "identf": identf, "identb": identb,
        "convw": convw, "convb": convb, "jkw": jkw, "jkb": jkb,
        "bns": bns, "bnt": bnt, "l1w": l1w, "l1b": l1b, "l2w": l2w, "l2b": l2b,
    }

    dst_core = core_of[dst]
    dst_off = off[dst]
    src_tab = tab[src]

    in_maps = []
    for c in range(NC):
        eidx = np.flatnonzero(dst_core == c)
        e_win = dst_off[eidx] // 128
        e_k = src_tab[eidx] // CHROWS
        key = e_win * NCH + e_k
        order = np.argsort(key, kind="stable")
        eidx = eidx[order]
        key = key[order]
        counts = np.bincount(key, minlength=NW * NCH)
        assert (counts <= CH * 128).all(), f"chunk overflow core {c}"
        starts = np.concatenate([[0], np.cumsum(counts)])[:-1]
        pos = np.arange(len(eidx)) - starts[key]
        # slot space: [NW, NCH, CH, 128]
        slot = key * (CH * 128) + pos

        idx_slots = np.zeros(NW * CPW * 128, dtype=np.int64)
        rel_slots = np.zeros(NW * CPW * 128, dtype=np.float32)
        nrm_slots = np.zeros(NW * CPW * 128, dtype=np.float32)
        idx_slots[slot] = src_tab[eidx] % CHROWS
        rel_slots[slot] = (dst_off[eidx] % 128).astype(np.float32)
        nrm_slots[slot] = norm[eidx]

        # gather idx runs: [NW, NCH, NIDX]
        runs = idx_slots.reshape(NW, NCH, NIDX)
        gidx = _wrap_idxs(runs.reshape(NW * NCH, NIDX))  # [128, NW*NCH*ICOLS]
        rel_cols = rel_slots.reshape(NW * CPW, 128).T.copy()
        nrm_cols = nrm_slots.reshape(NW * CPW, 128).T.copy()

        # per-node columns
        d2 = np.zeros((128, NW), dtype=np.float32)
        ln = np.arange(sizes[c], dtype=np.int64)
        d2[ln % 128, ln // 128] = dinv2[bounds[c] + ln]
        pool = np.zeros((128, NW * GPC), dtype=ml_dtypes.bfloat16)
        g_of = batch[bounds[c] + ln].astype(np.int64) - c * GPC
        pool[ln % 128, (ln // 128) * GPC + g_of] = 1.0

        x_nm = np.zeros((S, F), dtype=ml_dtypes.bfloat16)
        x_nm[: sizes[c]] = x[bounds[c]: bounds[c + 1]].astype(ml_dtypes.bfloat16)

        m = {"x_nm": x_nm, "gidx": gidx, "rel": rel_cols, "norm": nrm_cols,
             "dinv2": d2, "pool": pool}
        m.update(shared)
        in_maps.append(m)
    return in_maps


def _build_program(stage=99):
    nc = bacc.Bacc("TRN2", target_bir_lowering=False, debug=False,
                   num_devices=NC)
    AF = mybir.ActivationFunctionType
    OP = mybir.AluOpType

    ap = {}
    for name, shape, dt in [
        ("x_nm", [S, F], bf16), ("xtab", [TAB, F], bf16),
        ("gidx", [128, NW * NCH * ICOLS], i16),
        ("rel", [128, NW * CPW], f32), ("norm", [128, NW * CPW], f32),
        ("dinv2", [128, NW], f32), ("pool", [128, NW * GPC], bf16),
        ("iota", [128, 128], bf16), ("identf", [128, 128], f32),
        ("identb", [128, 128], bf16),
        ("convw", [F, 6 * F], bf16), ("convb", [F, 6], f32),
        ("jkw", [F, 6 * F], bf16), ("jkb", [F, NB], f32),
        ("bns", [F, NB], f32), ("bnt", [F, NB], f32),
        ("l1w", [F, NB * F], f32), ("l1b", [F, 1], f32),
        ("l2w", [F, 10], f32), ("l2b", [10, 1], f32),
    ]:
        ap[name] = nc.dram_tensor(name, shape, dt, kind="ExternalInput").ap()
    out_ap = nc.dram_tensor("out", [GPC, 10], f32, kind="ExternalOutput").ap()

    with tile.TileContext(nc) as tc:
        with (
            tc.tile_pool(name="dram", bufs=1, space="DRAM") as dram,
            tc.tile_pool(name="pers", bufs=1) as pers,
            tc.tile_pool(name="rot", bufs=1) as rot,
            tc.tile_pool(name="psum", bufs=1, space="PSUM") as psum,
        ):
            ag_in = dram.tile([S, F], bf16)

            # ---- persistent SBUF loads
            sb = {}
            for name in ["gidx", "rel", "norm", "dinv2", "pool", "iota",
                         "identf", "identb", "convw", "convb", "jkw", "jkb",
                         "bns", "bnt", "l1w", "l1b", "l2w", "l2b"]:
                t_ = pers.tile(list(ap[name].shape), ap[name].dtype, name=f"sb_{name}")
                nc.sync.dma_start(t_[:], ap[name][:])
                sb[name] = t_

            h_nm = pers.tile([128, NW, F], bf16, name="h_nm")

            h1_fm = pers.tile([128, S], bf16, name="h1_fm")
            h2_fm = pers.tile([128, S], bf16, name="h2_fm")
            hb_fm = pers.tile([128, S], bf16, name="hb_fm")
            z_sb = pers.tile([128, NB, GPC], f32, name="z_sb")

            agg_sb = pers.tile([128, NW, F], f32, name="agg_sb")

            def conv(lk, tables, h_out, write_nm):
                for k in range(NCH):
                    for w in range(NWRUN):
                        G = rot.tile([128, CH, F], bf16, tag="G", bufs=_bufs(4), name="G")
                        nc.gpsimd.dma_gather(
                            out_ap=G[:], in_ap=tables[k][:],
                            idxs_ap=sb["gidx"][:, (w * NCH + k) * ICOLS:
                                               (w * NCH + k + 1) * ICOLS],
                            num_idxs=NIDX, num_idxs_reg=NIDX, elem_size=F,
                            single_packet=False)
                        pp = psum.tile([128, F], f32, tag="agg", bufs=_bufs(2), name="pp")
                        for c in range(CH):
                            col = w * CPW + k * CH + c
                            oh = rot.tile([128, 128], bf16, tag="oh", bufs=_bufs(6), name="oh")
                            nc.vector.tensor_scalar(
                                out=oh[:], in0=sb["iota"][:],
                                scalar1=sb["rel"][:, col:col + 1],
                                scalar2=sb["norm"][:, col:col + 1],
                                op0=OP.is_equal, op1=OP.mult)
                            nc.tensor.matmul(pp[:], oh[:], G[:, c, :],
                                             start=(c == 0), stop=(c == CH - 1))
                        if k == 0:
                            nc.vector.tensor_copy(agg_sb[:, w, :], pp[:])
                        else:
                            nc.vector.tensor_tensor(out=agg_sb[:, w, :],
                                                    in0=agg_sb[:, w, :], in1=pp[:],
                                                    op=OP.add)
                for w in range(NWRUN):
                    t_nm = rot.tile([128, F], f32, tag="tnm", bufs=_bufs(3), name="t_nm")
                    nc.vector.tensor_scalar_mul(t_nm[:], h_nm[:, w, :],
                                                sb["dinv2"][:, w:w + 1])
                    nc.vector.tensor_tensor(out=t_nm[:], in0=t_nm[:],
                                            in1=agg_sb[:, w, :], op=OP.add)
                    tT = psum.tile([128, F], f32, tag="tT", bufs=_bufs(2), name="tT")
                    nc.tensor.transpose(tT[:], t_nm[:], sb["identf"][:])
                    tTs = rot.tile([128, F], bf16, tag="tTs", bufs=_bufs(3), name="tTs")
                    nc.scalar.copy(tTs[:], tT[:])
                    hn = psum.tile([128, F], f32, tag="hn", bufs=_bufs(2), name="hn")
                    nc.tensor.matmul(hn[:], sb["convw"][:, lk * F:(lk + 1) * F],
                                     tTs[:], start=True, stop=True)
                    nc.scalar.activation(h_out[:, w * 128:(w + 1) * 128], hn[:],
                                         AF.Relu, bias=sb["convb"][:, lk:lk + 1])
                    if write_nm:
                        hnT = psum.tile([128, F], bf16, tag="hnT", bufs=1, name="hnT")
                        nc.tensor.transpose(hnT[:], h_out[:, w * 128:(w + 1) * 128],
                                            sb["identb"][:])
                        nc.scalar.copy(h_nm[:, w, :], hnT[:])
                        nc.sync.dma_start(ag_in[w * 128:(w + 1) * 128, :],
                                          h_nm[:, w, :])

            def allgather(i):
                tabs = []
                for k in range(NCH):
                    tk = dram.tile([CHROWS, F], bf16, addr_space="Shared",
                                   tag=f"t{_rep[0]}_{i}_{k}",
                                   name=f"t{_rep[0]}_{i}_{k}")
                    nc.gpsimd.collective_compute(
                        "AllGather", OP.bypass,
                        replica_groups=[list(range(NC))],
                        ins=[ag_in[k * CHS:(k + 1) * CHS, :].opt()],
                        outs=[tk.opt()])
                    tabs.append(tk)
                return tabs

            def jk(li, last):
                pooled = psum.tile([128, GPC], f32, tag="pooled", bufs=1,
                                   name="pooled")
                for w in range(NW):
                    hb = psum.tile([128, F], f32, tag="hn", bufs=_bufs(2), name="hb")
                    nc.tensor.matmul(hb[:], sb["jkw"][:, (2 * li) * F:(2 * li + 1) * F],
                                     h1_fm[:, w * 128:(w + 1) * 128],
                                     start=True, stop=False)
                    nc.tensor.matmul(hb[:], sb["jkw"][:, (2 * li + 1) * F:(2 * li + 2) * F],
                                     h2_fm[:, w * 128:(w + 1) * 128],
                                     start=False, stop=True)
                    nc.scalar.activation(hb_fm[:, w * 128:(w + 1) * 128], hb[:],
                                         AF.Relu, bias=sb["jkb"][:, li:li + 1])
                    hnT = psum.tile([128, F], bf16, tag="hnT", bufs=1, name="hnT")
                    nc.tensor.transpose(hnT[:], hb_fm[:, w * 128:(w + 1) * 128],
                                        sb["identb"][:])
                    nc.scalar.copy(h_nm[:, w, :], hnT[:])
                    if not last:
                        nc.sync.dma_start(ag_in[w * 128:(w + 1) * 128, :],
                                          h_nm[:, w, :])
                    nc.tensor.matmul(pooled[:], h_nm[:, w, :],
                                     sb["pool"][:, w * GPC:(w + 1) * GPC],
                                     start=(w == 0), stop=(w == NW - 1))
                nc.scalar.copy(z_sb[:, li, :], pooled[:])

            # ---- main flow
            _rep = [0]
            steps = [
                lambda: conv(0, [ap["xtab"][k * CHROWS:(k + 1) * CHROWS, :] for k in range(NCH)], h1_fm, True),
                lambda: allgather(0),
                lambda t: conv(1, t, h2_fm, False),
                lambda: jk(0, False),
                lambda: allgather(1),
                lambda t: conv(2, t, h1_fm, True),
                lambda: allgather(2),
                lambda t: conv(3, t, h2_fm, False),
                lambda: jk(1, False),
                lambda: allgather(3),
                lambda t: conv(4, t, h1_fm, True),
                lambda: allgather(4),
                lambda t: conv(5, t, h2_fm, False),
                lambda: jk(2, True),
            ]
            for rep in range(REPEAT):
                _rep[0] = rep
                for w in range(NW):
                    nc.sync.dma_start(h_nm[:, w, :],
                                      ap["x_nm"][w * 128:(w + 1) * 128, :])
                table = None
                for i, st in enumerate(steps):
                    if i >= stage:
                        break
                    r = st(table) if st.__code__.co_argcount else st()
                    if r is not None:
                        table = r

            # ---- head
            if stage < 14:
                outt0 = rot.tile([GPC, 10], f32, tag="outt", bufs=1, name="outt0")
                nc.vector.tensor_copy(outt0[:], h1_fm[0:GPC, 0:10])
                nc.sync.dma_start(out_ap[:], outt0[:])
            else:
                _head(nc, tc, rot, psum, sb, z_sb, out_ap)

    nc.compile()
    return nc


def _head(nc, tc, rot, psum, sb, z_sb, out_ap):
    AF = mybir.ActivationFunctionType
    OP = mybir.AluOpType
    if True:
        if True:
            zbn = rot.tile([128, NB, GPC], f32, tag="zbn", bufs=1, name="zbn")
            for t in range(NB):
                nc.vector.tensor_scalar(
                    out=zbn[:, t, :], in0=z_sb[:, t, :],
                    scalar1=sb["bns"][:, t:t + 1], scalar2=sb["bnt"][:, t:t + 1],
                    op0=OP.mult, op1=OP.add)
            a1 = psum.tile([128, GPC], f32, tag="hn", bufs=_bufs(2), name="a1")
            for t in range(NB):
                nc.tensor.matmul(a1[:], sb["l1w"][:, t * F:(t + 1) * F],
                                 zbn[:, t, :], start=(t == 0), stop=(t == NB - 1))
            a1s = rot.tile([128, GPC], f32, tag="a1s", bufs=1, name="a1s")
            nc.scalar.activation(a1s[:], a1[:], AF.Relu, bias=sb["l1b"][:])
            z2 = psum.tile([10, GPC], f32, tag="pooled", bufs=1, name="z2")
            nc.tensor.matmul(z2[:], sb["l2w"][:], a1s[:], start=True, stop=True)
            z2s = rot.tile([10, GPC], f32, tag="z2s", bufs=1, name="z2s")
            nc.scalar.activation(z2s[:], z2[:], AF.Identity, bias=sb["l2b"][:])
            z2T = psum.tile([GPC, 10], f32, tag="tT", bufs=_bufs(2), name="z2T")
            nc.tensor.transpose(z2T[:], z2s[:], sb["identf"][0:10, 0:10])
            z2Ts = rot.tile([GPC, 10], f32, tag="z2Ts", bufs=1, name="z2Ts")
            nc.vector.tensor_copy(z2Ts[:], z2T[:])
            negm = rot.tile([GPC, 1], f32, tag="negm", bufs=1, name="negm")
            nc.vector.tensor_reduce(negm[:], z2Ts[:], mybir.AxisListType.X,
                                    OP.max, negate=True)
            et = rot.tile([GPC, 10], f32, tag="et", bufs=1, name="et")
            nc.scalar.activation(et[:], z2Ts[:], AF.Exp, bias=negm[:])
            ssum = rot.tile([GPC, 1], f32, tag="ssum", bufs=1, name="ssum")
            nc.vector.tensor_reduce(ssum[:], et[:], mybir.AxisListType.X, OP.add)
            rcp = rot.tile([GPC, 1], f32, tag="rcp", bufs=1, name="rcp")
            nc.vector.reciprocal(rcp[:], ssum[:])
            outt = rot.tile([GPC, 10], f32, tag="outt", bufs=1, name="outt")
            nc.vector.tensor_scalar_mul(outt[:], et[:], rcp[:])
            nc.sync.dma_start(out_ap[:], outt[:])


def _get_program():
    global _PROGRAM
    if _PROGRAM is None:
        _PROGRAM = _build_program()
    return _PROGRAM


def kernel(**inputs) -> np.ndarray:
    in_maps = _preprocess(inputs)
    nc = _get_program()
    res = run_bass_kernel_spmd(nc, in_maps, list(range(NC)))
    return np.concatenate([res.results[c]["out"] for c in range(NC)], axis=0)

